# revision 23
# baseline (speedup 1.0000x reference)
"""MLA (multi-head latent attention) forward on 8 TRN2 NeuronCores.

Strategy: tensor-parallel over heads (16 heads -> 2 per core). Every core
runs the identical Bass/Tile program on its own head-slice of the weights:
  P1  per batch: kv latent projection (+layernorm, +rope) and q projection
      (+rope, +scale) from a single streaming pass over x^T; the latent kv
      is kept in SBUF in both [k,c] and [c,k] orientations (bf16), or staged
      through DRAM (f32/f32r).
  P3  per batch: causal attention per head in "scores transposed" [k,q]
      orientation (no PE transposes of attention weights needed); softmax
      denominator via ones-vector matmul; normalization folded into the
      latent-context eviction via a PE broadcast of 1/sum.
  P4  output projection with the per-core wo column slice; host sums the
      8 partial outputs.

MM_MODE selects the matmul operand dtype: "f32" (exact, 4 PE cycles/row),
"bf16" (1 cycle/row, ~3e-3 rel err).
"""

import numpy as np
from contextlib import ExitStack

import concourse.bass as bass
import concourse.tile as tile
from concourse import bacc, mybir

# ---------------- problem dims (hardcoded per contest contract) -------------
B, S, D = 2, 2048, 2048
H = 16
C = 512            # kv_lora_rank
DN, DR, DV = 128, 64, 128
NH = DN + DR       # 192
SCALE = float((DN + DR) ** -0.5)
NEG = -1e9

N_CORES = 8
HL = H // N_CORES  # 2 local heads
P = 128
DC = D // P        # 16 contraction chunks over D
TCH = S // P       # 16 token chunks per batch
NBLK = 4           # query blocks per batch
BLKQ = S // NBLK   # 512
CC = C // P        # 4 chunks over latent dim
XG = 4             # token chunks per x-block DMA

F32 = mybir.dt.float32
AF = mybir.ActivationFunctionType
ALU = mybir.AluOpType
AX = mybir.AxisListType

MM_MODE = "bf16"   # "f32" | "bf16" | "f32r"


def _mmdt():
    return {"f32": mybir.dt.float32,
            "bf16": mybir.dt.bfloat16,
            "f32r": mybir.dt.float32r}[MM_MODE]


def _npdt():
    return np.float32 if MM_MODE != "bf16" else __import__("ml_dtypes").bfloat16


def _mm(nc, out, lhsT, rhs, start, stop):
    nc.tensor.matmul(out, lhsT, rhs, start=start, stop=stop,
                     skip_group_check=True)


def build_nc():
    from concourse.masks import make_identity

    MDT = _mmdt()
    assert MM_MODE == "bf16", "this build keeps latent kv in SBUF (bf16 only)"
    nc = bacc.Bacc("TRN2", target_bir_lowering=False, debug=False,
                   num_devices=N_CORES)
    xt = nc.dram_tensor("xt", [D, B * S], MDT, kind="ExternalInput")
    wqt = nc.dram_tensor("wqt", [D, HL * NH], MDT, kind="ExternalInput")
    wkvat = nc.dram_tensor("wkvat", [D, C + DR], MDT, kind="ExternalInput")
    wkc = nc.dram_tensor("wkc", [HL, DN, C], MDT, kind="ExternalInput")
    wvct = nc.dram_tensor("wvct", [HL, C, DV], MDT, kind="ExternalInput")
    wot = nc.dram_tensor("wot", [HL * DV, D], MDT, kind="ExternalInput")
    cs = nc.dram_tensor("cs", [S, DR], F32, kind="ExternalInput")
    maskt = nc.dram_tensor("maskt", [CC, P, BLKQ], F32, kind="ExternalInput")
    eshift = nc.dram_tensor("eshift", [P, 1], F32, kind="ExternalInput")
    out = nc.dram_tensor("out", [B * S, D], F32, kind="ExternalOutput")

    with tile.TileContext(nc) as tc, ExitStack() as ctx:
        const = ctx.enter_context(tc.tile_pool(name="const", bufs=1))
        ident = const.tile([P, P], F32, name="ident", tag="ident")
        make_identity(nc, ident)
        ones_col = const.tile([P, 1], MDT, name="ones_col", tag="ones_col")
        nc.vector.memset(ones_col, 1.0)
        ones_row = const.tile([1, P], F32, name="ones_row", tag="ones_row")
        nc.vector.memset(ones_row, 1.0)
        cs_sb = const.tile([P, TCH, DR], F32, name="cs_sb", tag="cs_sb")
        nc.sync.dma_start(out=cs_sb,
                          in_=cs[:, :].rearrange("(a p) r -> p a r", p=P))
        mask_sb = const.tile([P, CC, BLKQ], F32, name="mask_sb", tag="mask_sb")
        nc.sync.dma_start(out=mask_sb,
                          in_=maskt[:, :, :].rearrange("m p n -> p m n"))
        wkc_sb = const.tile([P, HL, C], MDT, name="wkc_sb", tag="wkc_sb")
        nc.sync.dma_start(out=wkc_sb,
                          in_=wkc[:, :, :].rearrange("h p c -> p h c"))
        wvct_sb = const.tile([P, HL, CC, DV], MDT, name="wvct_sb", tag="wvct_sb")
        nc.sync.dma_start(
            out=wvct_sb,
            in_=wvct[:, :, :].rearrange("h (cc p) v -> p h cc v", p=P))
        esh_sb = const.tile([P, 1], F32, name="esh_sb", tag="esh_sb")
        nc.sync.dma_start(out=esh_sb, in_=eshift[:, :])
        eps_sb = const.tile([P, 1], F32, name="eps_sb", tag="eps_sb")
        nc.vector.memset(eps_sb, 1e-5)

        octpool = ctx.enter_context(tc.tile_pool(name="octpool", bufs=1))
        outcT = [octpool.tile([P, B * S], MDT, name=f"outcT{hl}",
                              tag=f"outcT{hl}")
                 for hl in range(HL)]
        wot_sb = [octpool.tile([P, D], MDT, name=f"wot_sb{hl}", tag=f"wot{hl}")
                  for hl in range(HL)]
        for hl in range(HL):
            nc.scalar.dma_start(out=wot_sb[hl], in_=wot[hl * P:(hl + 1) * P, :])
        wq_sb = octpool.tile([P, DC, HL * NH], MDT, name="wq_sb", tag="wq_sb")
        nc.sync.dma_start(out=wq_sb,
                          in_=wqt[:, :].rearrange("(a p) n -> p a n", p=P))
        wkva_sb = octpool.tile([P, DC, C + DR], MDT, name="wkva_sb",
                               tag="wkva_sb")
        nc.scalar.dma_start(out=wkva_sb,
                            in_=wkvat[:, :].rearrange("(a p) n -> p a n", p=P))

        for b in range(B):
            with ExitStack() as bctx:
                bper = bctx.enter_context(tc.tile_pool(name=f"bper{b}", bufs=1))
                nopeT = [bper.tile([P, S], MDT, name=f"nopeT{b}{h}",
                                   tag=f"nopeT{h}")
                         for h in range(HL)]
                peT = [bper.tile([DR, S], MDT, name=f"peT{b}{h}", tag=f"peT{h}")
                       for h in range(HL)]
                kpeT = bper.tile([DR, S], MDT, name=f"kpeT{b}", tag="kpeT")
                # latent kv resident in SBUF, both orientations
                kvcT = bper.tile([P, CC, S], MDT, name=f"kvcT{b}", tag="kvcT")
                kvnat = bper.tile([P, TCH, C], MDT, name=f"kvnat{b}",
                                  tag="kvnat")

                # ---------------- P1: projections ----------------
                with ExitStack() as p1:
                    xpool = p1.enter_context(tc.tile_pool(name=f"xp{b}", bufs=2))
                    kvfpool = p1.enter_context(tc.tile_pool(name=f"kvf{b}",
                                                            bufs=2))
                    qnpool = p1.enter_context(tc.tile_pool(name=f"qn{b}", bufs=2))
                    sqpool = p1.enter_context(tc.tile_pool(name=f"sq{b}", bufs=2))
                    tmp = p1.enter_context(tc.tile_pool(name=f"tmp{b}", bufs=4))
                    kvps = p1.enter_context(
                        tc.tile_pool(name=f"kvps{b}", bufs=2, space="PSUM"))
                    qps = p1.enter_context(
                        tc.tile_pool(name=f"qps{b}", bufs=2, space="PSUM"))
                    tps = p1.enter_context(
                        tc.tile_pool(name=f"tps{b}", bufs=2, space="PSUM"))

                    def emit_transposes(tch, kvf, qn):
                        tok0 = tch * P
                        for cc in range(CC):
                            tp = tps.tile([P, P], F32, name="tp", tag="tp")
                            nc.tensor.transpose(
                                tp, kvf[:, cc * P:(cc + 1) * P], ident)
                            nc.vector.tensor_copy(
                                kvcT[:, cc, tok0:tok0 + P], tp)
                        tpk = tps.tile([P, P], F32, name="tpk", tag="tp")
                        nc.tensor.transpose(tpk[0:DR, :], kvf[:, C:C + DR],
                                            ident)
                        nc.vector.tensor_copy(kpeT[:, tok0:tok0 + P],
                                              tpk[0:DR, :])
                        for h in range(HL):
                            o = h * NH + DN
                            tq = tps.tile([P, P], F32, name="tq", tag="tp")
                            nc.tensor.transpose(
                                tq, qn[:, h * NH:h * NH + DN], ident)
                            nc.vector.tensor_copy(nopeT[h][:, tok0:tok0 + P],
                                                  tq)
                            tq2 = tps.tile([P, P], F32, name="tq2", tag="tp")
                            nc.tensor.transpose(tq2[0:DR, :], qn[:, o:o + DR],
                                                ident)
                            nc.vector.tensor_copy(peT[h][:, tok0:tok0 + P],
                                                  tq2[0:DR, :])

                    deferred = None
                    for tg in range(TCH // XG):
                        xblk = xpool.tile([P, DC, XG * P], MDT, name="xblk",
                                          tag="xblk")
                        g0 = b * S + tg * XG * P
                        nc.sync.dma_start(
                            out=xblk,
                            in_=xt[:, g0:g0 + XG * P]
                                .rearrange("(a p) t -> p a t", p=P))
                        for ti in range(XG):
                            tch = tg * XG + ti
                            tok0 = tch * P
                            xv = xblk[:, :, ti * P:(ti + 1) * P]

                            # ---- latent kv projection ----
                            kv0 = kvps.tile([P, 288], F32, name="kv0", tag="kv0")
                            kv1 = kvps.tile([P, 288], F32, name="kv1", tag="kv1")
                            for dc in range(DC):
                                _mm(nc, kv0, xv[:, dc], wkva_sb[:, dc, 0:288],
                                    start=(dc == 0), stop=(dc == DC - 1))
                            for dc in range(DC):
                                _mm(nc, kv1, xv[:, dc], wkva_sb[:, dc, 288:576],
                                    start=(dc == 0), stop=(dc == DC - 1))
                            # ---- q projection ----
                            qp = qps.tile([P, HL * NH], F32, name="qp", tag="qp")
                            for dc in range(DC):
                                _mm(nc, qp, xv[:, dc], wq_sb[:, dc],
                                    start=(dc == 0), stop=(dc == DC - 1))
                            # transposes of the PREVIOUS chunk (its LN/rope
                            # has had a full chunk of time to finish)
                            if deferred is not None:
                                emit_transposes(*deferred)

                            kvf = kvfpool.tile([P, C + DR], F32, name="kvf",
                                               tag="kvf")
                            nc.scalar.copy(kvf[:, 0:288], kv0)
                            nc.scalar.copy(kvf[:, 288:576], kv1)

                            # ---- layernorm over latent channels ----
                            msum = tmp.tile([P, 1], F32, name="msum", tag="msum")
                            nc.vector.tensor_reduce(msum, kvf[:, 0:C],
                                                    axis=AX.X, op=ALU.add)
                            mneg = tmp.tile([P, 1], F32, name="mneg", tag="mneg")
                            nc.scalar.mul(mneg, msum, -1.0 / C)
                            nc.vector.tensor_scalar_add(kvf[:, 0:C],
                                                        kvf[:, 0:C], mneg)
                            sq = sqpool.tile([P, C], F32, name="sq", tag="sq")
                            nc.scalar.square(sq, kvf[:, 0:C])
                            var = tmp.tile([P, 1], F32, name="var", tag="var")
                            nc.vector.tensor_reduce(var, sq, axis=AX.X,
                                                    op=ALU.add)
                            std = tmp.tile([P, 1], F32, name="std", tag="std")
                            nc.scalar.activation(std, var, AF.Sqrt,
                                                 bias=eps_sb, scale=1.0 / C)
                            rstd = tmp.tile([P, 1], F32, name="rstd",
                                            tag="rstd")
                            nc.vector.reciprocal(rstd, std)
                            nc.vector.tensor_scalar_mul(kvf[:, 0:C],
                                                        kvf[:, 0:C], rstd)

                            # ---- rope on shared key pe ----
                            kp = kvf[:, C:C + DR].rearrange(
                                "p (i two) -> p i two", two=2)
                            ke, ko = kp[:, :, 0], kp[:, :, 1]
                            cosv = cs_sb[:, tch, 0:DR // 2]
                            sinv = cs_sb[:, tch, DR // 2:DR]
                            t1 = tmp.tile([P, DR // 2], F32, name="t1", tag="t1")
                            t2 = tmp.tile([P, DR // 2], F32, name="t2", tag="t2")
                            t3 = tmp.tile([P, DR // 2], F32, name="t3", tag="t3")
                            t4 = tmp.tile([P, DR // 2], F32, name="t4", tag="t4")
                            nc.vector.tensor_mul(t1, ke, cosv)
                            nc.vector.tensor_mul(t2, ko, sinv)
                            nc.vector.tensor_mul(t3, ke, sinv)
                            nc.vector.tensor_mul(t4, ko, cosv)
                            nc.vector.tensor_sub(ke, t1, t2)
                            nc.vector.tensor_add(ko, t3, t4)

                            # ---- store latent kv natural (rounded) ----
                            nc.vector.tensor_copy(kvnat[:, tch, :],
                                                  kvf[:, 0:C])

                            # ---- scale + rope the q projection ----
                            qn = qnpool.tile([P, HL * NH], F32, name="qn",
                                             tag="qn")
                            nc.scalar.mul(qn, qp, SCALE)
                            for h in range(HL):
                                o = h * NH + DN
                                qpe = qn[:, o:o + DR].rearrange(
                                    "p (i two) -> p i two", two=2)
                                qe, qo = qpe[:, :, 0], qpe[:, :, 1]
                                u1 = tmp.tile([P, DR // 2], F32, name="u1",
                                              tag="u1")
                                u2 = tmp.tile([P, DR // 2], F32, name="u2",
                                              tag="u2")
                                u3 = tmp.tile([P, DR // 2], F32, name="u3",
                                              tag="u3")
                                u4 = tmp.tile([P, DR // 2], F32, name="u4",
                                              tag="u4")
                                nc.vector.tensor_mul(u1, qe, cosv)
                                nc.vector.tensor_mul(u2, qo, sinv)
                                nc.vector.tensor_mul(u3, qe, sinv)
                                nc.vector.tensor_mul(u4, qo, cosv)
                                nc.vector.tensor_sub(qe, u1, u2)
                                nc.vector.tensor_add(qo, u3, u4)
                            deferred = (tch, kvf, qn)
                    emit_transposes(*deferred)

                # ---------------- P3: attention ----------------
                with ExitStack() as p3:
                    qabs_p = p3.enter_context(tc.tile_pool(name=f"qab{b}",
                                                           bufs=2))
                    ex_p = p3.enter_context(tc.tile_pool(name=f"ex{b}", bufs=6))
                    ctx_p = p3.enter_context(tc.tile_pool(name=f"ctxp{b}",
                                                          bufs=2))
                    sm_p = p3.enter_context(tc.tile_pool(name=f"smp{b}", bufs=2))
                    ctxu_p = p3.enter_context(tc.tile_pool(name=f"ctxu{b}",
                                                           bufs=2))
                    psA = p3.enter_context(
                        tc.tile_pool(name=f"psA{b}", bufs=3, space="PSUM"))
                    psC = p3.enter_context(
                        tc.tile_pool(name=f"psC{b}", bufs=1, space="PSUM"))
                    psS = p3.enter_context(
                        tc.tile_pool(name=f"psS{b}", bufs=1, space="PSUM"))

                    for h in range(HL):
                        # q_absT for the whole head, one 4x512-wide pass
                        qabsT_h = qabs_p.tile([P, CC, S], MDT,
                                              name="qabsT", tag="qabsT")
                        for qg in range(NBLK):
                            for cc in range(CC):
                                qa = psA.tile([P, BLKQ], F32, name="qa",
                                              tag="psA")
                                _mm(nc, qa, wkc_sb[:, h, cc * P:(cc + 1) * P],
                                    nopeT[h][:, qg * BLKQ:(qg + 1) * BLKQ],
                                    start=True, stop=True)
                                nc.scalar.copy(
                                    qabsT_h[:, cc, qg * BLKQ:(qg + 1) * BLKQ],
                                    qa)
                        for blk in range(NBLK):
                            nkc = (blk + 1) * (BLKQ // P)
                            q0 = blk * BLKQ
                            qsl = slice(q0, q0 + BLKQ)
                            qabsT = qabsT_h[:, :, qsl]
                            ctxp = psC.tile([P, CC, BLKQ], F32,
                                            name="ctxp", tag="psC")
                            sump = psS.tile([1, BLKQ], F32, name="sump",
                                            tag="psS")

                            def consume(ex, kc):
                                _mm(nc, sump, ones_col, ex,
                                    start=(kc == 0), stop=(kc == nkc - 1))
                                kvn_ap = kvnat[:, kc, :]
                                for cc in range(CC):
                                    _mm(nc, ctxp[:, cc],
                                        kvn_ap[:, cc * P:(cc + 1) * P], ex,
                                        start=(kc == 0), stop=(kc == nkc - 1))

                            pending = None
                            for kc in range(nkc):
                                k0 = kc * P
                                kvt_ap = kvcT[:, :, k0:k0 + P]
                                sp = psA.tile([P, BLKQ], F32, name="sp",
                                              tag="psA")
                                for cc in range(CC):
                                    _mm(nc, sp, kvt_ap[:, cc], qabsT[:, cc],
                                        start=(cc == 0), stop=False)
                                _mm(nc, sp, kpeT[:, k0:k0 + P], peT[h][:, qsl],
                                    start=False, stop=True)
                                if kc >= nkc - CC:
                                    br = kc - (nkc - CC)
                                    nc.vector.tensor_add(sp, sp, mask_sb[:, br])
                                ex = ex_p.tile([P, BLKQ], MDT, name="ex",
                                               tag="ex")
                                nc.scalar.activation(ex, sp, AF.Exp,
                                                     bias=esh_sb)
                                if pending is not None:
                                    consume(*pending)
                                pending = (ex, kc)
                            consume(*pending)
                            # evict ctx psum immediately to free its banks for
                            # the next block's k-loop; normalize from SBUF
                            ctxu = ctxu_p.tile([P, CC, BLKQ], F32,
                                               name="ctxu", tag="ctxu")
                            for cc in range(CC):
                                if cc % 2 == 0:
                                    nc.vector.tensor_copy(ctxu[:, cc],
                                                          ctxp[:, cc])
                                else:
                                    nc.scalar.copy(ctxu[:, cc], ctxp[:, cc])
                            recip = sm_p.tile([1, BLKQ], F32,
                                              name="recip", tag="recip")
                            nc.vector.reciprocal(recip, sump)
                            bcp = psA.tile([P, BLKQ], F32, name="bcp",
                                           tag="psA")
                            nc.tensor.matmul(bcp, ones_row, recip,
                                             start=True, stop=True,
                                             skip_group_check=True)
                            bcs = sm_p.tile([P, BLKQ], F32, name="bcs",
                                            tag="bcs")
                            nc.vector.tensor_copy(bcs, bcp)
                            ctxs = ctx_p.tile([P, CC, BLKQ], MDT,
                                              name="ctxs", tag="ctxs")
                            for cc in range(CC):
                                nc.vector.tensor_mul(ctxs[:, cc], ctxu[:, cc],
                                                     bcs)
                            ocp = psA.tile([P, BLKQ], F32, name="ocp",
                                           tag="psA")
                            for cc in range(CC):
                                _mm(nc, ocp, wvct_sb[:, h, cc], ctxs[:, cc],
                                    start=(cc == 0), stop=(cc == CC - 1))
                            nc.scalar.copy(
                                outcT[h][:, b * S + q0:b * S + q0 + BLKQ], ocp)

            # ---------------- P4: output projection for this batch ---------
            with ExitStack() as p4:
                o_pool = p4.enter_context(tc.tile_pool(name=f"op{b}", bufs=3))
                psO = p4.enter_context(
                    tc.tile_pool(name=f"psO{b}", bufs=4, space="PSUM"))
                for qc in range(b * TCH, (b + 1) * TCH):
                    osb = o_pool.tile([P, D], F32, name="osb", tag="osb")
                    for dg in range(D // 512):
                        op = psO.tile([P, 512], F32, name="op", tag="psO")
                        for hl in range(HL):
                            _mm(nc, op, outcT[hl][:, qc * P:(qc + 1) * P],
                                wot_sb[hl][:, dg * 512:(dg + 1) * 512],
                                start=(hl == 0), stop=(hl == HL - 1))
                        if dg % 2 == 0:
                            nc.vector.tensor_copy(
                                osb[:, dg * 512:(dg + 1) * 512], op)
                        else:
                            nc.scalar.copy(osb[:, dg * 512:(dg + 1) * 512], op)
                    nc.sync.dma_start(out=out[qc * P:(qc + 1) * P, :], in_=osb)
    nc.finalize()
    return nc


_cache = {}


def get_nc():
    key = MM_MODE
    if key not in _cache:
        _cache[key] = build_nc()
    return _cache[key]


def make_in_maps(x, wq, wkv_a, kv_g, kv_b, wkv_b, wo, start_pos):
    """Host-side sharding/layout prep. Returns (in_maps, out_bias)."""
    x = np.asarray(x, dtype=np.float32)
    wq = np.asarray(wq, dtype=np.float32)
    wkv_a = np.asarray(wkv_a, dtype=np.float32)
    kv_g = np.asarray(kv_g, dtype=np.float32)
    kv_b = np.asarray(kv_b, dtype=np.float32)
    wkv_b = np.asarray(wkv_b, dtype=np.float32)
    wo = np.asarray(wo, dtype=np.float32)
    sp = int(start_pos)
    ndt = _npdt()

    xt = np.ascontiguousarray(x.reshape(B * S, D).T.astype(ndt))

    pos = (sp + np.arange(S)).astype(np.float32)
    inv = 1.0 / (10000.0 ** (np.arange(0, DR, 2, dtype=np.float32) / DR))
    ang = pos[:, None] * inv
    cs = np.concatenate([np.cos(ang), np.sin(ang)], axis=1).astype(np.float32)

    kk = np.arange(P, dtype=np.int64)
    qq = np.arange(BLKQ, dtype=np.int64)
    maskt = np.zeros((CC, P, BLKQ), np.float32)
    for br in range(CC):
        keys = br * P + kk
        maskt[br] = np.where(keys[:, None] <= qq[None, :], 0.0, NEG)

    wkvb = wkv_b.reshape(H, DN + DV, C)
    # fold layernorm gamma into the absorbed projections; beta contributes a
    # softmax-invariant score shift plus a constant output bias added on host
    wkc_all = wkvb[:, :DN, :] * kv_g[None, None, :]
    wvc_all = wkvb[:, DN:, :] * kv_g[None, None, :]
    bias_hv = (wkvb[:, DN:, :] @ kv_b).reshape(H * DV)
    out_bias = (bias_hv @ wo.T).astype(np.float32)

    eshift = np.zeros((P, 1), np.float32)

    in_maps = []
    for c in range(N_CORES):
        hs = slice(HL * c, HL * (c + 1))
        wq_h = wq.reshape(H, NH, D)[hs].reshape(HL * NH, D)
        in_maps.append({
            "xt": xt,
            "wqt": np.ascontiguousarray(wq_h.T.astype(ndt)),
            "wkvat": np.ascontiguousarray(wkv_a.T.astype(ndt)),
            "wkc": np.ascontiguousarray(wkc_all[hs].astype(ndt)),
            "wvct": np.ascontiguousarray(
                np.swapaxes(wvc_all[hs], 1, 2).astype(ndt)),
            "wot": np.ascontiguousarray(
                wo[:, HL * DV * c:HL * DV * (c + 1)].T.astype(ndt)),
            "cs": cs,
            "maskt": maskt,
            "eshift": eshift,
        })
    return in_maps, out_bias


def kernel(x, wq, wkv_a, kv_g, kv_b, wkv_b, wo, start_pos):
    from concourse.bass_utils import run_bass_kernel_spmd

    in_maps, out_bias = make_in_maps(x, wq, wkv_a, kv_g, kv_b, wkv_b, wo,
                                     start_pos)
    res = run_bass_kernel_spmd(get_nc(), in_maps, list(range(N_CORES)))
    acc = np.zeros((B * S, D), np.float64)
    for r in res.results:
        acc += r["out"]
    acc += out_bias[None, :]
    return acc.astype(np.float32).reshape(B, S, D)
